# revision 23
# baseline (speedup 1.0000x reference)
"""PersLay forward on 8 Trainium2 NeuronCores.

Computation: k[p, m] = exp(-2*|points[p] - theta[m]|^2), feats = segment_sum(k),
out = feats @ fc_w.T + fc_b.

Strategy:
  - Each core owns 256 contiguous segments (segment_ids are sorted, so each
    core's points are a contiguous range -> pure data parallel, no collectives).
  - Host repacks points into per-segment slots: each segment's points are split
    into two halves living at the same columns of partition blocks 0-63 (theta
    copy A) and 64-127 (theta copy B), so all 128 lanes are busy.
  - Slots are rank-scheduled: each core sorts its 256 half-segments by size
    (descending); rank r across all cores shares one slot width W_r =
    max_core(size of rank-r half-segment), rounded up to a multiple of 8.
    Consecutive ranks pack into equal-width chunks (chunk cols <= 2048 = one
    4-bank PSUM tile), so padding is tiny and the SPMD program is identical
    across cores (per-core raggedness lives in the data).
  - logits[j, t] = 4*theta_x*x + 4*theta_y*y - 2*(x^2+y^2) via a K=16 bf16
    matmul: each fp32 factor is split hi+lo into two bf16 values (a*x ~=
    ah*xh + ah*xl + al*xh, exact to ~1e-3 in the logits) because native fp32
    matmul runs in the slow LOW_HIGH two-pass mode on TRN2. -2*|theta|^2 goes
    into the exp bias.
  - exp is split across engines to beat the ScalarE throughput wall:
    A-chunks use ScalarE table exp (exact); B-chunks use a Schraudolph
    bit-trick exp on VectorE: uint32(logit*(2^23/ln2) + C) bitcast to fp32,
    with the fp32->uint32 store saturating negatives to 0 (so the padding
    and underflowed tails become exactly +0.0). C is tuned on a host sample
    to zero the mean error (~+-3% sawtooth per element, ~0 bias over sums).
  - Segment sum: fold1 (add the two halves of every slot, 3D APs) on GpSimd
    or VectorE per a static plan, fold2 + 3D tensor_reduce on VectorE.
  - Host inverts the rank permutation, folds the two partition halves, and
    applies the tiny FC layer.
Padding columns carry r2 = 1e30 so exp maps them to exactly 0.
"""

import numpy as np

NCORES = 8
NSEG = 2048
M = 64
PAD_R2 = 1.0e30
SCH_A = 12102203.161561485  # 2^23 / ln 2

# chunk plan tuning
SCH_EVERY = 6        # every SCH_EVERY-th chunk uses Schraudolph-on-DVE
DVE_FOLD1_EVERY = 4  # among A-chunks, every n-th keeps fold1 on DVE


def _ensure_concourse():
    try:
        import concourse  # noqa: F401
    except ImportError:
        import sys

        for p in ("/opt/trn_rl_repo", "/root/.axon_site/_ro/trn_rl_repo"):
            if p not in sys.path:
                sys.path.insert(0, p)


def _schedule(halves):
    """Build the shared chunk schedule from per-core sorted half-segment sizes.

    halves: [NSEG] per-segment half sizes. Returns (chunks, order) where
    chunks = [(n_slots, W)] and order[core, r] = local segment index assigned
    to rank-r slot.
    """
    b_per = NSEG // NCORES
    h = halves.reshape(NCORES, b_per)
    order = np.argsort(-h, axis=1, kind="stable")          # rank -> local seg
    sorted_h = np.take_along_axis(h, order, axis=1)
    rank_w = sorted_h.max(axis=0)                          # [b_per]
    rank_w = np.maximum((rank_w + 7) // 8 * 8, 8).astype(np.int64)

    chunks = []
    r = 0
    while r < b_per:
        w = int(rank_w[r])
        n = min(2048 // w, b_per - r)
        chunks.append((n, w))
        r += n
    # split the first chunk so the pipeline primes on a small unit
    n0, w0 = chunks[0]
    if n0 > 2:
        chunks = [(2, w0), (n0 - 2, w0)] + chunks[1:]
    return chunks, order


def _plan(chunks):
    """Assign per-chunk exp engine and fold1 engine."""
    return [("A", "vector")] * len(chunks)


def _group_chunks(chunks):
    """DMA batches: single chunks first (fast pipeline fill), then fours."""
    sizes = [1, 1, 1, 1, 2, 2]
    groups = []
    i = 0
    while i < len(chunks):
        size = sizes[len(groups)] if len(groups) < len(sizes) else 4
        groups.append(chunks[i:i + size])
        i += size
    return groups


def _build_program(chunks, sch_c):
    import concourse.bass as bass
    import concourse.tile as tile
    from concourse import bacc, mybir

    n_slot = sum(n for n, _ in chunks)
    total_cols = sum(n * w for n, w in chunks)
    plan = _plan(chunks)

    nc = bacc.Bacc("TRN2", target_bir_lowering=False, debug=False,
                   num_devices=1, enable_asserts=False)
    bg = nc.dram_tensor("bg", [16, total_cols], mybir.dt.bfloat16,
                        kind="ExternalInput").ap()
    a2 = nc.dram_tensor("a2", [16, 128], mybir.dt.bfloat16,
                        kind="ExternalInput").ap()
    bias = nc.dram_tensor("bias", [128, 1], mybir.dt.float32,
                          kind="ExternalInput").ap()
    biasb = nc.dram_tensor("biasb", [128, 1], mybir.dt.float32,
                           kind="ExternalInput").ap()
    feats_out = nc.dram_tensor("feats", [128, n_slot], mybir.dt.float32,
                               kind="ExternalOutput").ap()

    groups = _group_chunks(chunks)
    max_group_cols = max(sum(n * w for n, w in g) for g in groups)

    with tile.TileContext(nc) as tc:
        with (
            tc.tile_pool(name="const", bufs=1) as const_pool,
            tc.tile_pool(name="work", bufs=1) as work_pool,
            tc.tile_pool(name="ps", bufs=1, space=bass.MemorySpace.PSUM) as ps_pool,
        ):
            # Warm the exp table before any data arrives (ACT_TABLE_LOAD is
            # emitted before the first Exp; a dummy op hoists it off the
            # critical path).
            dummy_t = const_pool.tile([1, 8], mybir.dt.float16)
            with tc.high_priority():
                nc.scalar.activation(dummy_t[:], dummy_t[:],
                                     mybir.ActivationFunctionType.Exp)
            a_t = const_pool.tile([16, 128], mybir.dt.bfloat16)
            nc.sync.dma_start(a_t[:], a2[:])
            feats_t = const_pool.tile([128, n_slot], mybir.dt.float32)

            big_b = [work_pool.tile([16, max_group_cols], mybir.dt.bfloat16,
                                    name=f"bigb{i}", tag=f"bigb{i}")
                     for i in range(3)]
            ps = [ps_pool.tile([128, 2048], mybir.dt.float32, name=f"ps{i}",
                               tag=f"ps{i}") for i in range(2)]
            k_t = [work_pool.tile([128, 2048], mybir.dt.float16,
                                  name=f"kt{i}", tag=f"kt{i}")
                   for i in range(3)]
            nb = sum(1 for m, _ in plan if m == "B")
            kb_t = [work_pool.tile([128, 2048], mybir.dt.uint32,
                                   name=f"kbt{i}", tag=f"kbt{i}")
                    for i in range(min(nb, 2))]
            f1_t = [work_pool.tile([128, 1024], mybir.dt.float16,
                                   name=f"f1{i}", tag=f"f1{i}")
                    for i in range(3)]
            f2_t = [work_pool.tile([128, 512], mybir.dt.float16,
                                   name=f"f2{i}", tag=f"f2{i}")
                    for i in range(3)]
            f1b_t = [work_pool.tile([128, 1024], mybir.dt.float32,
                                    name=f"f1b{i}", tag=f"f1b{i}")
                     for i in range(min(nb, 2))]
            f2b_t = [work_pool.tile([128, 512], mybir.dt.float32,
                                    name=f"f2b{i}", tag=f"f2b{i}")
                     for i in range(min(nb, 2))]

            col = 0
            slot = 0
            ci = 0
            bi = 0
            bias_t = None
            biasb_t = None
            for gi, g in enumerate(groups):
                gcols = sum(n * w for n, w in g)
                bb = big_b[gi % 3]
                nc.sync.dma_start(bb[:, 0:gcols], bg[:, col:col + gcols])
                if gi == 0:
                    # After the first input chunk is in flight: small consts
                    # needed only by the (later) first ACT.
                    bias_t = const_pool.tile([128, 1], mybir.dt.float32)
                    nc.sync.dma_start(bias_t[:], bias[:])
                    biasb_t = const_pool.tile([128, 1], mybir.dt.float32)
                    nc.sync.dma_start(biasb_t[:], biasb[:])
                goff = 0
                for n, w in g:
                    cw = n * w
                    p = ps[ci % 2]
                    for j in range(0, cw, 512):
                        e = min(j + 512, cw)
                        nc.tensor.matmul(p[:, j:e], a_t[:],
                                         bb[:, goff + j:goff + e],
                                         start=True, stop=True)
                    mode, f1eng = plan[ci]
                    h1 = w // 2
                    h2 = w // 4
                    if mode == "A":
                        kt = k_t[ci % 3]
                        nc.scalar.activation(kt[:, 0:cw], p[:, 0:cw],
                                             mybir.ActivationFunctionType.Exp,
                                             bias=bias_t[:], scale=1.0)
                        k3 = kt[:, 0:cw].rearrange("p (n w) -> p n w", w=w)
                        f1 = f1_t[ci % 3][:, 0:n * h1].rearrange(
                            "p (n w) -> p n w", w=h1)
                        eng = nc.vector if f1eng == "vector" else nc.gpsimd
                        eng.tensor_tensor(f1, k3[:, :, 0:h1], k3[:, :, h1:w],
                                          mybir.AluOpType.add)
                        f2 = f2_t[ci % 3][:, 0:n * h2].rearrange(
                            "p (n w) -> p n w", w=h2)
                        nc.vector.tensor_add(f2, f1[:, :, 0:h2],
                                             f1[:, :, h2:h1])
                        nc.vector.reduce_sum(feats_t[:, slot:slot + n], f2,
                                             axis=mybir.AxisListType.X)
                    else:
                        kb = kb_t[bi % 2]
                        nc.vector.tensor_scalar(
                            kb[:, 0:cw], p[:, 0:cw], float(SCH_A),
                            biasb_t[:], mybir.AluOpType.mult,
                            mybir.AluOpType.add)
                        kf = kb[:, 0:cw].bitcast(mybir.dt.float32)
                        k3 = kf.rearrange("p (n w) -> p n w", w=w)
                        f1 = f1b_t[bi % 2][:, 0:n * h1].rearrange(
                            "p (n w) -> p n w", w=h1)
                        nc.vector.tensor_add(f1, k3[:, :, 0:h1],
                                             k3[:, :, h1:w])
                        f2 = f2b_t[bi % 2][:, 0:n * h2].rearrange(
                            "p (n w) -> p n w", w=h2)
                        nc.vector.tensor_add(f2, f1[:, :, 0:h2],
                                             f1[:, :, h2:h1])
                        nc.vector.reduce_sum(feats_t[:, slot:slot + n], f2,
                                             axis=mybir.AxisListType.X)
                        bi += 1
                    goff += cw
                    slot += n
                    ci += 1
                col += gcols
            nc.sync.dma_start(feats_out[:], feats_t[:])

    nc.compile()
    return nc


def _split_bf16(v):
    import ml_dtypes

    hi = v.astype(ml_dtypes.bfloat16)
    lo = (v - hi.astype(np.float32)).astype(ml_dtypes.bfloat16)
    return hi, lo


def _tune_sch_c(points, theta):
    """Pick the Schraudolph additive constant C that zeroes the mean error
    of sum(exp) over a sample of the actual logit distribution."""
    rng = np.random.default_rng(12345)
    idx = rng.choice(points.shape[0], size=4096, replace=False)
    p = points[idx].astype(np.float64)
    th = theta.astype(np.float64)
    d2 = ((p[:, None, :] - th[None, :, :]) ** 2).sum(-1)
    logits = np.clip(-2.0 * d2, -200.0, 0.0).ravel()
    true_sum = np.exp(logits).sum()
    a = np.float32(SCH_A)
    lf = logits.astype(np.float32)
    best = None
    for c in np.linspace(1064500000.0, 1065353216.0, 48):
        y = lf * a + np.float32(c)
        i = np.where(y > 0, np.rint(y), 0).astype(np.uint32)
        s = i.view(np.float32).astype(np.float64).sum()
        err = abs(s - true_sum)
        if best is None or err < best[0]:
            best = (err, float(c))
    return best[1]


def _prepare_inputs(points, segment_ids):
    """Repack [P, 2] points into per-core [16, total_cols] bf16 slot arrays.

    Unique value rows per half: xh, xl, yh, yl, r2h, r2l; expanded to the
    8-row K pattern [xh, xl, xh, yh, yl, yh, r2h, r2l] that pairs with the
    stationary rows [ah_x, ah_x, al_x, ah_y, ah_y, al_y, -2, -2].
    """
    import ml_dtypes

    points = np.ascontiguousarray(points, dtype=np.float32)
    seg = np.asarray(segment_ids).astype(np.int64).ravel()
    p_total = points.shape[0]
    b_per = NSEG // NCORES

    counts = np.bincount(seg, minlength=NSEG)
    starts = np.zeros(NSEG, np.int64)
    np.cumsum(counts[:-1], out=starts[1:])
    halves = (counts + 1) // 2
    chunks, order = _schedule(halves)

    n_slot = sum(n for n, _ in chunks)
    total_cols = sum(n * w for n, w in chunks)
    # rank -> starting column of its slot
    rank_col = np.zeros(n_slot, np.int64)
    c = 0
    r = 0
    for n, w in chunks:
        rank_col[r:r + n] = c + np.arange(n) * w
        c += n * w
        r += n
    # local segment -> rank (invert order per core)
    seg_rank = np.empty((NCORES, b_per), np.int64)
    np.put_along_axis(seg_rank, order, np.arange(b_per)[None, :], axis=1)

    r_pt = np.arange(p_total, dtype=np.int64) - starts[seg]   # rank in segment
    hs = halves[seg]
    first = r_pt < hs
    col_in_slot = np.where(first, r_pt, r_pt - hs)
    half = np.where(first, 0, 1)
    core = seg >> 8  # 256 segments per core
    local_col = rank_col[seg_rank[core, seg & 255]] + col_in_slot

    x = points[:, 0]
    y = points[:, 1]
    r2 = x * x + y * y
    xh, xl = _split_bf16(x)
    yh, yl = _split_bf16(y)
    r2h, r2l = _split_bf16(r2)

    bf = ml_dtypes.bfloat16
    u = np.zeros((NCORES, 2, 6, total_cols), bf)
    u[:, :, 4, :] = bf(PAD_R2)  # padding: r2 = huge -> exp(-2r2) = 0
    u[core, half, 0, local_col] = xh
    u[core, half, 1, local_col] = xl
    u[core, half, 2, local_col] = yh
    u[core, half, 3, local_col] = yl
    u[core, half, 4, local_col] = r2h
    u[core, half, 5, local_col] = r2l
    expand = [0, 1, 0, 2, 3, 2, 4, 5]
    bg = np.ascontiguousarray(
        u[:, :, expand, :].reshape(NCORES, 16, total_cols))
    return bg, chunks, seg_rank


def _theta_consts(theta, sch_c):
    import ml_dtypes

    theta = np.asarray(theta, dtype=np.float32)
    ax = 4.0 * theta[:, 0]
    ay = 4.0 * theta[:, 1]
    ahx, alx = _split_bf16(ax)
    ahy, aly = _split_bf16(ay)
    a2 = np.zeros((16, 128), ml_dtypes.bfloat16)
    for blk, (j0, j1) in enumerate(((0, 64), (64, 128))):
        o = 8 * blk
        a2[o + 0, j0:j1] = ahx
        a2[o + 1, j0:j1] = ahx
        a2[o + 2, j0:j1] = alx
        a2[o + 3, j0:j1] = ahy
        a2[o + 4, j0:j1] = ahy
        a2[o + 5, j0:j1] = aly
        a2[o + 6, j0:j1] = ml_dtypes.bfloat16(-2.0)
        a2[o + 7, j0:j1] = ml_dtypes.bfloat16(-2.0)
    th2 = -2.0 * (theta[:, 0] ** 2 + theta[:, 1] ** 2)
    bias = np.concatenate([th2, th2]).reshape(128, 1).astype(np.float32)
    # Schraudolph: u32(logit*A + (C + A*bias)) per partition
    biasb = (np.float32(sch_c)
             + np.float32(SCH_A) * bias.astype(np.float32)).astype(np.float32)
    return a2, bias, biasb


def _run(points, segment_ids, theta, fc_w, fc_b, trace=False,
         trace_cores=None):
    _ensure_concourse()
    from concourse.bass_utils import run_bass_kernel_spmd

    points = np.ascontiguousarray(points, dtype=np.float32)
    theta = np.asarray(theta, dtype=np.float32)
    bg, chunks, seg_rank = _prepare_inputs(points, segment_ids)
    sch_c = _tune_sch_c(points, theta)
    a2, bias, biasb = _theta_consts(theta, sch_c)
    nc = _build_program(chunks, sch_c)

    in_maps = [{"bg": bg[c], "a2": a2, "bias": bias, "biasb": biasb}
               for c in range(NCORES)]
    res = run_bass_kernel_spmd(nc, in_maps, list(range(NCORES)), trace=trace,
                               trace_cores=trace_cores)

    b_per = NSEG // NCORES
    f = np.stack([res.results[c]["feats"] for c in range(NCORES)])
    f = f[:, :64, :] + f[:, 64:128, :]                     # fold theta copies
    # f[core, m, rank] -> feats[core, local_seg, m] via rank permutation
    core_idx = np.arange(NCORES)[:, None]
    feats = f[core_idx, :, seg_rank].reshape(NSEG, M)
    fc_w = np.asarray(fc_w, dtype=np.float32)
    fc_b = np.asarray(fc_b, dtype=np.float32)
    out = feats @ fc_w.T + fc_b
    return out.astype(np.float32), res


def kernel(points, segment_ids, theta, fc_w, fc_b):
    out, _ = _run(points, segment_ids, theta, fc_w, fc_b, trace=False)
    return out


# revision 24
# speedup vs baseline: 1.0145x; 1.0145x over previous
"""PersLay forward on 8 Trainium2 NeuronCores.

Computation: k[p, m] = exp(-2*|points[p] - theta[m]|^2), feats = segment_sum(k),
out = feats @ fc_w.T + fc_b.

Strategy:
  - Each core owns 256 contiguous segments (segment_ids are sorted, so each
    core's points are a contiguous range -> pure data parallel, no collectives).
  - Host repacks points into per-segment slots: each segment's points are split
    into two halves living at the same columns of partition blocks 0-63 (theta
    copy A) and 64-127 (theta copy B), so all 128 lanes are busy.
  - Slots are rank-scheduled: each core sorts its 256 half-segments by size
    (descending); rank r across all cores shares one slot width W_r =
    max_core(size of rank-r half-segment), rounded up to a multiple of 8.
    Consecutive ranks pack into equal-width chunks (chunk cols <= 2048 = one
    4-bank PSUM tile), so padding is tiny and the SPMD program is identical
    across cores (per-core raggedness lives in the data).
  - logits[j, t] = 4*theta_x*x + 4*theta_y*y - 2*(x^2+y^2) via a K=16 bf16
    matmul: each fp32 factor is split hi+lo into two bf16 values (a*x ~=
    ah*xh + ah*xl + al*xh, exact to ~1e-3 in the logits) because native fp32
    matmul runs in the slow LOW_HIGH two-pass mode on TRN2. -2*|theta|^2 goes
    into the exp bias.
  - exp is split across engines to beat the ScalarE throughput wall:
    A-chunks use ScalarE table exp (exact); B-chunks use a Schraudolph
    bit-trick exp on VectorE: uint32(logit*(2^23/ln2) + C) bitcast to fp32,
    with the fp32->uint32 store saturating negatives to 0 (so the padding
    and underflowed tails become exactly +0.0). C is tuned on a host sample
    to zero the mean error (~+-3% sawtooth per element, ~0 bias over sums).
  - Segment sum: fold1 (add the two halves of every slot, 3D APs) on GpSimd
    or VectorE per a static plan, fold2 + 3D tensor_reduce on VectorE.
  - Host inverts the rank permutation, folds the two partition halves, and
    applies the tiny FC layer.
Padding columns carry r2 = 1e30 so exp maps them to exactly 0.
"""

import numpy as np

NCORES = 8
NSEG = 2048
M = 64
PAD_R2 = 1.0e30
SCH_A = 12102203.161561485  # 2^23 / ln 2

# chunk plan tuning
SCH_EVERY = 6        # every SCH_EVERY-th chunk uses Schraudolph-on-DVE
DVE_FOLD1_EVERY = 4  # among A-chunks, every n-th keeps fold1 on DVE


def _ensure_concourse():
    try:
        import concourse  # noqa: F401
    except ImportError:
        import sys

        for p in ("/opt/trn_rl_repo", "/root/.axon_site/_ro/trn_rl_repo"):
            if p not in sys.path:
                sys.path.insert(0, p)


def _schedule(halves):
    """Build the shared chunk schedule from per-core sorted half-segment sizes.

    halves: [NSEG] per-segment half sizes. Returns (chunks, order) where
    chunks = [(n_slots, W)] and order[core, r] = local segment index assigned
    to rank-r slot.
    """
    b_per = NSEG // NCORES
    h = halves.reshape(NCORES, b_per)
    order = np.argsort(-h, axis=1, kind="stable")          # rank -> local seg
    sorted_h = np.take_along_axis(h, order, axis=1)
    rank_w = sorted_h.max(axis=0)                          # [b_per]
    rank_w = np.maximum((rank_w + 7) // 8 * 8, 8).astype(np.int64)

    chunks = []
    r = 0
    while r < b_per:
        w = int(rank_w[r])
        n = min(2048 // w, b_per - r)
        chunks.append((n, w))
        r += n
    return chunks, order


def _plan(chunks):
    """Assign per-chunk exp engine and fold1 engine."""
    return [("A", "vector")] * len(chunks)


def _group_chunks(chunks):
    """DMA batches: single chunks first (fast pipeline fill), then fours."""
    sizes = [1, 1, 1, 1, 2, 2]
    groups = []
    i = 0
    while i < len(chunks):
        size = sizes[len(groups)] if len(groups) < len(sizes) else 4
        groups.append(chunks[i:i + size])
        i += size
    return groups


def _build_program(chunks, sch_c):
    import concourse.bass as bass
    import concourse.tile as tile
    from concourse import bacc, mybir

    n_slot = sum(n for n, _ in chunks)
    total_cols = sum(n * w for n, w in chunks)
    plan = _plan(chunks)

    nc = bacc.Bacc("TRN2", target_bir_lowering=False, debug=False,
                   num_devices=1, enable_asserts=False)
    bg = nc.dram_tensor("bg", [16, total_cols], mybir.dt.bfloat16,
                        kind="ExternalInput").ap()
    a2 = nc.dram_tensor("a2", [16, 128], mybir.dt.bfloat16,
                        kind="ExternalInput").ap()
    bias = nc.dram_tensor("bias", [128, 1], mybir.dt.float32,
                          kind="ExternalInput").ap()
    biasb = nc.dram_tensor("biasb", [128, 1], mybir.dt.float32,
                           kind="ExternalInput").ap()
    feats_out = nc.dram_tensor("feats", [128, n_slot], mybir.dt.float32,
                               kind="ExternalOutput").ap()

    groups = _group_chunks(chunks)
    max_group_cols = max(sum(n * w for n, w in g) for g in groups)

    with tile.TileContext(nc) as tc:
        with (
            tc.tile_pool(name="const", bufs=1) as const_pool,
            tc.tile_pool(name="work", bufs=1) as work_pool,
            tc.tile_pool(name="ps", bufs=1, space=bass.MemorySpace.PSUM) as ps_pool,
        ):
            # Warm the exp table before any data arrives (ACT_TABLE_LOAD is
            # emitted before the first Exp; a dummy op hoists it off the
            # critical path).
            dummy_t = const_pool.tile([1, 8], mybir.dt.float16)
            with tc.high_priority():
                nc.scalar.activation(dummy_t[:], dummy_t[:],
                                     mybir.ActivationFunctionType.Exp)
            a_t = const_pool.tile([16, 128], mybir.dt.bfloat16)
            nc.sync.dma_start(a_t[:], a2[:])
            feats_t = const_pool.tile([128, n_slot], mybir.dt.float32)

            big_b = [work_pool.tile([16, max_group_cols], mybir.dt.bfloat16,
                                    name=f"bigb{i}", tag=f"bigb{i}")
                     for i in range(3)]
            ps = [ps_pool.tile([128, 2048], mybir.dt.float32, name=f"ps{i}",
                               tag=f"ps{i}") for i in range(2)]
            k_t = [work_pool.tile([128, 2048], mybir.dt.float16,
                                  name=f"kt{i}", tag=f"kt{i}")
                   for i in range(3)]
            nb = sum(1 for m, _ in plan if m == "B")
            kb_t = [work_pool.tile([128, 2048], mybir.dt.uint32,
                                   name=f"kbt{i}", tag=f"kbt{i}")
                    for i in range(min(nb, 2))]
            f1_t = [work_pool.tile([128, 1024], mybir.dt.float16,
                                   name=f"f1{i}", tag=f"f1{i}")
                    for i in range(3)]
            f2_t = [work_pool.tile([128, 512], mybir.dt.float16,
                                   name=f"f2{i}", tag=f"f2{i}")
                    for i in range(3)]
            f1b_t = [work_pool.tile([128, 1024], mybir.dt.float32,
                                    name=f"f1b{i}", tag=f"f1b{i}")
                     for i in range(min(nb, 2))]
            f2b_t = [work_pool.tile([128, 512], mybir.dt.float32,
                                    name=f"f2b{i}", tag=f"f2b{i}")
                     for i in range(min(nb, 2))]

            col = 0
            slot = 0
            ci = 0
            bi = 0
            bias_t = None
            biasb_t = None
            for gi, g in enumerate(groups):
                gcols = sum(n * w for n, w in g)
                bb = big_b[gi % 3]
                nc.sync.dma_start(bb[:, 0:gcols], bg[:, col:col + gcols])
                if gi == 0:
                    # After the first input chunk is in flight: small consts
                    # needed only by the (later) first ACT.
                    bias_t = const_pool.tile([128, 1], mybir.dt.float32)
                    nc.sync.dma_start(bias_t[:], bias[:])
                    biasb_t = const_pool.tile([128, 1], mybir.dt.float32)
                    nc.sync.dma_start(biasb_t[:], biasb[:])
                goff = 0
                for n, w in g:
                    cw = n * w
                    p = ps[ci % 2]
                    for j in range(0, cw, 512):
                        e = min(j + 512, cw)
                        nc.tensor.matmul(p[:, j:e], a_t[:],
                                         bb[:, goff + j:goff + e],
                                         start=True, stop=True)
                    mode, f1eng = plan[ci]
                    h1 = w // 2
                    h2 = w // 4
                    if mode == "A":
                        kt = k_t[ci % 3]
                        nc.scalar.activation(kt[:, 0:cw], p[:, 0:cw],
                                             mybir.ActivationFunctionType.Exp,
                                             bias=bias_t[:], scale=1.0)
                        k3 = kt[:, 0:cw].rearrange("p (n w) -> p n w", w=w)
                        f1 = f1_t[ci % 3][:, 0:n * h1].rearrange(
                            "p (n w) -> p n w", w=h1)
                        eng = nc.vector if f1eng == "vector" else nc.gpsimd
                        eng.tensor_tensor(f1, k3[:, :, 0:h1], k3[:, :, h1:w],
                                          mybir.AluOpType.add)
                        f2 = f2_t[ci % 3][:, 0:n * h2].rearrange(
                            "p (n w) -> p n w", w=h2)
                        nc.vector.tensor_add(f2, f1[:, :, 0:h2],
                                             f1[:, :, h2:h1])
                        nc.vector.reduce_sum(feats_t[:, slot:slot + n], f2,
                                             axis=mybir.AxisListType.X)
                    else:
                        kb = kb_t[bi % 2]
                        nc.vector.tensor_scalar(
                            kb[:, 0:cw], p[:, 0:cw], float(SCH_A),
                            biasb_t[:], mybir.AluOpType.mult,
                            mybir.AluOpType.add)
                        kf = kb[:, 0:cw].bitcast(mybir.dt.float32)
                        k3 = kf.rearrange("p (n w) -> p n w", w=w)
                        f1 = f1b_t[bi % 2][:, 0:n * h1].rearrange(
                            "p (n w) -> p n w", w=h1)
                        nc.vector.tensor_add(f1, k3[:, :, 0:h1],
                                             k3[:, :, h1:w])
                        f2 = f2b_t[bi % 2][:, 0:n * h2].rearrange(
                            "p (n w) -> p n w", w=h2)
                        nc.vector.tensor_add(f2, f1[:, :, 0:h2],
                                             f1[:, :, h2:h1])
                        nc.vector.reduce_sum(feats_t[:, slot:slot + n], f2,
                                             axis=mybir.AxisListType.X)
                        bi += 1
                    goff += cw
                    slot += n
                    ci += 1
                col += gcols
            nc.sync.dma_start(feats_out[:], feats_t[:])

    nc.compile()
    return nc


def _split_bf16(v):
    import ml_dtypes

    hi = v.astype(ml_dtypes.bfloat16)
    lo = (v - hi.astype(np.float32)).astype(ml_dtypes.bfloat16)
    return hi, lo


def _tune_sch_c(points, theta):
    """Pick the Schraudolph additive constant C that zeroes the mean error
    of sum(exp) over a sample of the actual logit distribution."""
    rng = np.random.default_rng(12345)
    idx = rng.choice(points.shape[0], size=4096, replace=False)
    p = points[idx].astype(np.float64)
    th = theta.astype(np.float64)
    d2 = ((p[:, None, :] - th[None, :, :]) ** 2).sum(-1)
    logits = np.clip(-2.0 * d2, -200.0, 0.0).ravel()
    true_sum = np.exp(logits).sum()
    a = np.float32(SCH_A)
    lf = logits.astype(np.float32)
    best = None
    for c in np.linspace(1064500000.0, 1065353216.0, 48):
        y = lf * a + np.float32(c)
        i = np.where(y > 0, np.rint(y), 0).astype(np.uint32)
        s = i.view(np.float32).astype(np.float64).sum()
        err = abs(s - true_sum)
        if best is None or err < best[0]:
            best = (err, float(c))
    return best[1]


def _prepare_inputs(points, segment_ids):
    """Repack [P, 2] points into per-core [16, total_cols] bf16 slot arrays.

    Unique value rows per half: xh, xl, yh, yl, r2h, r2l; expanded to the
    8-row K pattern [xh, xl, xh, yh, yl, yh, r2h, r2l] that pairs with the
    stationary rows [ah_x, ah_x, al_x, ah_y, ah_y, al_y, -2, -2].
    """
    import ml_dtypes

    points = np.ascontiguousarray(points, dtype=np.float32)
    seg = np.asarray(segment_ids).astype(np.int64).ravel()
    p_total = points.shape[0]
    b_per = NSEG // NCORES

    counts = np.bincount(seg, minlength=NSEG)
    starts = np.zeros(NSEG, np.int64)
    np.cumsum(counts[:-1], out=starts[1:])
    halves = (counts + 1) // 2
    chunks, order = _schedule(halves)

    n_slot = sum(n for n, _ in chunks)
    total_cols = sum(n * w for n, w in chunks)
    # rank -> starting column of its slot
    rank_col = np.zeros(n_slot, np.int64)
    c = 0
    r = 0
    for n, w in chunks:
        rank_col[r:r + n] = c + np.arange(n) * w
        c += n * w
        r += n
    # local segment -> rank (invert order per core)
    seg_rank = np.empty((NCORES, b_per), np.int64)
    np.put_along_axis(seg_rank, order, np.arange(b_per)[None, :], axis=1)

    r_pt = np.arange(p_total, dtype=np.int64) - starts[seg]   # rank in segment
    hs = halves[seg]
    first = r_pt < hs
    col_in_slot = np.where(first, r_pt, r_pt - hs)
    half = np.where(first, 0, 1)
    core = seg >> 8  # 256 segments per core
    local_col = rank_col[seg_rank[core, seg & 255]] + col_in_slot

    x = points[:, 0]
    y = points[:, 1]
    r2 = x * x + y * y
    xh, xl = _split_bf16(x)
    yh, yl = _split_bf16(y)
    r2h, r2l = _split_bf16(r2)

    bf = ml_dtypes.bfloat16
    u = np.zeros((NCORES, 2, 6, total_cols), bf)
    u[:, :, 4, :] = bf(PAD_R2)  # padding: r2 = huge -> exp(-2r2) = 0
    u[core, half, 0, local_col] = xh
    u[core, half, 1, local_col] = xl
    u[core, half, 2, local_col] = yh
    u[core, half, 3, local_col] = yl
    u[core, half, 4, local_col] = r2h
    u[core, half, 5, local_col] = r2l
    expand = [0, 1, 0, 2, 3, 2, 4, 5]
    bg = np.ascontiguousarray(
        u[:, :, expand, :].reshape(NCORES, 16, total_cols))
    return bg, chunks, seg_rank


def _theta_consts(theta, sch_c):
    import ml_dtypes

    theta = np.asarray(theta, dtype=np.float32)
    ax = 4.0 * theta[:, 0]
    ay = 4.0 * theta[:, 1]
    ahx, alx = _split_bf16(ax)
    ahy, aly = _split_bf16(ay)
    a2 = np.zeros((16, 128), ml_dtypes.bfloat16)
    for blk, (j0, j1) in enumerate(((0, 64), (64, 128))):
        o = 8 * blk
        a2[o + 0, j0:j1] = ahx
        a2[o + 1, j0:j1] = ahx
        a2[o + 2, j0:j1] = alx
        a2[o + 3, j0:j1] = ahy
        a2[o + 4, j0:j1] = ahy
        a2[o + 5, j0:j1] = aly
        a2[o + 6, j0:j1] = ml_dtypes.bfloat16(-2.0)
        a2[o + 7, j0:j1] = ml_dtypes.bfloat16(-2.0)
    th2 = -2.0 * (theta[:, 0] ** 2 + theta[:, 1] ** 2)
    bias = np.concatenate([th2, th2]).reshape(128, 1).astype(np.float32)
    # Schraudolph: u32(logit*A + (C + A*bias)) per partition
    biasb = (np.float32(sch_c)
             + np.float32(SCH_A) * bias.astype(np.float32)).astype(np.float32)
    return a2, bias, biasb


def _run(points, segment_ids, theta, fc_w, fc_b, trace=False,
         trace_cores=None):
    _ensure_concourse()
    from concourse.bass_utils import run_bass_kernel_spmd

    points = np.ascontiguousarray(points, dtype=np.float32)
    theta = np.asarray(theta, dtype=np.float32)
    bg, chunks, seg_rank = _prepare_inputs(points, segment_ids)
    sch_c = _tune_sch_c(points, theta)
    a2, bias, biasb = _theta_consts(theta, sch_c)
    nc = _build_program(chunks, sch_c)

    in_maps = [{"bg": bg[c], "a2": a2, "bias": bias, "biasb": biasb}
               for c in range(NCORES)]
    res = run_bass_kernel_spmd(nc, in_maps, list(range(NCORES)), trace=trace,
                               trace_cores=trace_cores)

    b_per = NSEG // NCORES
    f = np.stack([res.results[c]["feats"] for c in range(NCORES)])
    f = f[:, :64, :] + f[:, 64:128, :]                     # fold theta copies
    # f[core, m, rank] -> feats[core, local_seg, m] via rank permutation
    core_idx = np.arange(NCORES)[:, None]
    feats = f[core_idx, :, seg_rank].reshape(NSEG, M)
    fc_w = np.asarray(fc_w, dtype=np.float32)
    fc_b = np.asarray(fc_b, dtype=np.float32)
    out = feats @ fc_w.T + fc_b
    return out.astype(np.float32), res


def kernel(points, segment_ids, theta, fc_w, fc_b):
    out, _ = _run(points, segment_ids, theta, fc_w, fc_b, trace=False)
    return out
